# revision 17
# baseline (speedup 1.0000x reference)
"""AUGRU cell (attention-scaled GRU update) on 8 Trainium2 NeuronCores.

Data-parallel: batch B=65536 sharded 8 ways (8192 rows/core); gate weights
replicated.  Per core (gate-major layout, batch on the free axis):

  gates_x = x @ W_x.T + b_x
  gates_h = h @ W_h.T + b_h
  u = sigmoid(U); r = sigmoid(R); t = tanh(Cx + r*Ch)
  h_new = h + att*u*(t - h)

v12 design -- 7 matmuls/group, ACT-bias sigmoids, R-first critical path:
  - biases enter via the ACT bias operand (per-partition [P,1]) -> no K=1
    bias-prefill matmuls.  sigU/sigR are one [P,1024] ACT op per pair.
  - R gate is computed FIRST (its sigmoid feeds m -> identity-matmul ->
    tanh, the longest chain); U follows; Ch before Cx so m is never gated
    behind the pcx WAR.
  - PSUM: pu pair [P,2,512] bufs=1 (2 banks) + pr pair bufs=1 (2) +
    pcx group bufs=2 (2) + pch group bufs=2 (2) = 8 banks; split U/R pools
    release matmul WARs right after each sigmoid instead of after both.
  - tanh per group emitted in the same stage as m/id so the Cx bank WAR
    releases early; identity matmul merges m = (Ch+bCh)*r into the open
    Cx bank (216ns on PE vs ~0.75us on DVE).
  - head: first x/h slices issued from the scalar queue in parallel with
    the sync queue's weight/attb DMAs; bulk of x/h as single 1.5MiB DMAs.
  - epilogue per pair (1024 cols): ua=att*u (DVE), d=t-h (GPSIMD),
    q=ua*d (DVE), ho=h+q (DVE), one output DMA; last pair split per group.
"""

import sys

sys.path.insert(0, "/opt/trn_rl_repo")

from contextlib import ExitStack

import numpy as np
import ml_dtypes

import concourse.bass as bass
import concourse.tile as tile
from concourse import bacc, mybir
from concourse.bass_utils import run_bass_kernel_spmd

F32 = mybir.dt.float32
BF16 = mybir.dt.bfloat16
AF = mybir.ActivationFunctionType
OP = mybir.AluOpType
BFNP = ml_dtypes.bfloat16

B = 65536
NCORES = 8
BL = B // NCORES  # 8192 rows per core
I = 128
H = 128
P = 128
ROWS = 512  # batch rows per group (one fp32 PSUM bank per gate)
NGROUPS = BL // ROWS  # 16
NP = NGROUPS // 2  # 8 pairs
PR = 2 * ROWS  # pair width 1024


def build_program():
    nc = bacc.Bacc("TRN2", target_bir_lowering=False, debug=False)

    xT_d = nc.dram_tensor("xT", [I, BL], BF16, kind="ExternalInput").ap()
    hT_d = nc.dram_tensor("hT", [H, BL], BF16, kind="ExternalInput").ap()
    ab_d = nc.dram_tensor("attb", [P, BL], BF16, kind="ExternalInput").ap()
    wx_d = nc.dram_tensor("wxT", [I, 3, P], BF16, kind="ExternalInput").ap()
    wh_d = nc.dram_tensor("whT", [H, 3, P], BF16, kind="ExternalInput").ap()
    bc_d = nc.dram_tensor("bcol", [P, 4], F32, kind="ExternalInput").ap()
    id_d = nc.dram_tensor("ident", [P, P], BF16, kind="ExternalInput").ap()
    o_d = nc.dram_tensor("h_newT", [H, BL], BF16, kind="ExternalOutput").ap()

    with tile.TileContext(nc) as tc, ExitStack() as ctx:
        consts = ctx.enter_context(tc.tile_pool(name="consts", bufs=1))
        io = ctx.enter_context(tc.tile_pool(name="io", bufs=1))
        gp = ctx.enter_context(tc.tile_pool(name="gp", bufs=2))
        ep = ctx.enter_context(tc.tile_pool(name="ep", bufs=3))
        pu = ctx.enter_context(tc.tile_pool(name="pu", bufs=1, space="PSUM"))
        pr = ctx.enter_context(tc.tile_pool(name="pr", bufs=1, space="PSUM"))
        pcx = ctx.enter_context(tc.tile_pool(name="pcx", bufs=2, space="PSUM"))
        pch = ctx.enter_context(tc.tile_pool(name="pch", bufs=1, space="PSUM"))

        # ---------------- one-time setup ----------------
        # whole-core input/attb tiles; DMAs fill column ranges
        xs = io.tile([P, BL], BF16, tag="xs")
        hs = io.tile([P, BL], BF16, tag="hs")
        ab = io.tile([P, BL], BF16, tag="ab")
        wT = consts.tile([P, 6, P], BF16, tag="wT")  # [xu, xr, xc, hu, hr, hc]
        bcol = consts.tile([P, 4], F32, tag="bcol")  # [bU, bR, bCx, bCh]
        ident = consts.tile([P, P], BF16, tag="ident")

        # scalar queue carries ZERO DMAs: its first instruction is the
        # implicit ACT_TABLE_LOAD, so the first sigmoid can fire as soon as
        # the first pair's matmuls retire.
        # sync HWDGE ring is FIFO: first-pair slices + weights first, then
        # pair-granular x/h through pair 3 in consumption order.
        nc.sync.dma_start(xs[:, 0:ROWS], xT_d[:, 0:ROWS])
        nc.sync.dma_start(hs[:, 0:ROWS], hT_d[:, 0:ROWS])
        nc.sync.dma_start(wT[:, 0:3, :], wx_d)
        nc.sync.dma_start(wT[:, 3:6, :], wh_d)
        nc.sync.dma_start(bcol, bc_d)
        nc.sync.dma_start(ident, id_d)
        nc.sync.dma_start(xs[:, ROWS:PR], xT_d[:, ROWS:PR])
        nc.sync.dma_start(hs[:, ROWS:PR], hT_d[:, ROWS:PR])
        for p in range(1, 4):
            lo, hi = p * PR, (p + 1) * PR
            nc.sync.dma_start(xs[:, lo:hi], xT_d[:, lo:hi])
            nc.sync.dma_start(hs[:, lo:hi], hT_d[:, lo:hi])
        # attb + the back half of x/h ride the idle GpSimd SWDGE queue
        nc.gpsimd.dma_start(ab[:, 0 : 2 * PR], ab_d[:, 0 : 2 * PR])
        nc.gpsimd.dma_start(xs[:, 4 * PR : 6 * PR], xT_d[:, 4 * PR : 6 * PR])
        nc.gpsimd.dma_start(hs[:, 4 * PR : 6 * PR], hT_d[:, 4 * PR : 6 * PR])
        nc.gpsimd.dma_start(ab[:, 2 * PR : 4 * PR], ab_d[:, 2 * PR : 4 * PR])
        nc.gpsimd.dma_start(xs[:, 6 * PR :], xT_d[:, 6 * PR :])
        nc.gpsimd.dma_start(hs[:, 6 * PR :], hT_d[:, 6 * PR :])
        nc.gpsimd.dma_start(ab[:, 4 * PR :], ab_d[:, 4 * PR :])

        stB = [None] * NP  # (u_ps, r_ps, cx0, cx1, ch0, ch1) per pair
        ups = [None] * NP
        tps = [None] * NP
        uas = [None] * NP

        def stage_b(p):
            sl0 = slice(2 * p * ROWS, (2 * p + 1) * ROWS)
            sl1 = slice((2 * p + 1) * ROWS, (2 * p + 2) * ROWS)
            xg = (xs[:, sl0], xs[:, sl1])
            hg = (hs[:, sl0], hs[:, sl1])
            u_ps = pu.tile([P, 2, ROWS], F32, tag="u_ps")
            r_ps = pr.tile([P, 2, ROWS], F32, tag="r_ps")
            # R first: its sigmoid heads the m -> id -> tanh chain
            nc.tensor.matmul(r_ps[:, 0, :], lhsT=wT[:, 1, :], rhs=xg[0], start=True, stop=False)
            nc.tensor.matmul(r_ps[:, 1, :], lhsT=wT[:, 1, :], rhs=xg[1], start=True, stop=False)
            nc.tensor.matmul(r_ps[:, 0, :], lhsT=wT[:, 4, :], rhs=hg[0], start=False, stop=True)
            nc.tensor.matmul(r_ps[:, 1, :], lhsT=wT[:, 4, :], rhs=hg[1], start=False, stop=True)
            nc.tensor.matmul(u_ps[:, 0, :], lhsT=wT[:, 0, :], rhs=xg[0], start=True, stop=False)
            nc.tensor.matmul(u_ps[:, 1, :], lhsT=wT[:, 0, :], rhs=xg[1], start=True, stop=False)
            nc.tensor.matmul(u_ps[:, 0, :], lhsT=wT[:, 3, :], rhs=hg[0], start=False, stop=True)
            nc.tensor.matmul(u_ps[:, 1, :], lhsT=wT[:, 3, :], rhs=hg[1], start=False, stop=True)
            ch = pch.tile([P, 2, ROWS], F32, tag="ch")
            cx0 = pcx.tile([P, ROWS], F32, tag="cx")
            cx1 = pcx.tile([P, ROWS], F32, tag="cx")
            nc.tensor.matmul(ch[:, 0, :], lhsT=wT[:, 5, :], rhs=hg[0], start=True, stop=True)
            nc.tensor.matmul(ch[:, 1, :], lhsT=wT[:, 5, :], rhs=hg[1], start=True, stop=True)
            nc.tensor.matmul(cx0, lhsT=wT[:, 2, :], rhs=xg[0], start=True, stop=False)  # stays open
            nc.tensor.matmul(cx1, lhsT=wT[:, 2, :], rhs=xg[1], start=True, stop=False)
            stB[p] = (u_ps, r_ps, cx0, cx1, ch)

        uq = [None] * (NP // 2)  # u quad tiles [P, 2(pair), 2(g), ROWS]
        tq = [None] * (NP // 2)

        def stage_c(p):
            u_ps, r_ps, cx0, cx1, ch = stB[p]
            qd, ph = p // 2, p % 2
            if p >= NP - 2:
                # last two pairs drain at pair/group granularity
                u = gp.tile([P, 2, ROWS], BF16, tag="u", name="upair")
                t = gp.tile([P, 2, ROWS], BF16, tag="t", name="tpair")
                uq[qd] = None
                ups[p], tps[p] = u, t
            else:
                if ph == 0:
                    uq[qd] = gp.tile([P, 2, 2, ROWS], BF16, tag="u", name="uquad")
                    tq[qd] = gp.tile([P, 2, 2, ROWS], BF16, tag="t", name="tquad")
                u, t = uq[qd][:, ph, :, :], tq[qd][:, ph, :, :]
            r = gp.tile([P, 2, ROWS], BF16, tag="r")
            m = gp.tile([P, 2, ROWS], BF16, tag="m")
            nc.scalar.activation(r, r_ps, AF.Sigmoid, bias=bcol[:, 1:2])
            # m per group: shortens the sigR -> m -> id -> tanh_g0 chain so
            # tanh_g0 is ready right as sigU retires (zero ACT bubble)
            nc.vector.scalar_tensor_tensor(
                m[:, 0, :], in0=ch[:, 0, :], scalar=bcol[:, 3:4], in1=r[:, 0, :],
                op0=OP.add, op1=OP.mult,
            )
            nc.tensor.matmul(cx0, lhsT=ident, rhs=m[:, 0, :], start=False, stop=True)
            nc.vector.scalar_tensor_tensor(
                m[:, 1, :], in0=ch[:, 1, :], scalar=bcol[:, 3:4], in1=r[:, 1, :],
                op0=OP.add, op1=OP.mult,
            )
            nc.tensor.matmul(cx1, lhsT=ident, rhs=m[:, 1, :], start=False, stop=True)
            nc.scalar.activation(u, u_ps, AF.Sigmoid, bias=bcol[:, 0:1])
            nc.scalar.activation(t[:, 0, :], cx0, AF.Tanh, bias=bcol[:, 2:3])
            nc.scalar.activation(t[:, 1, :], cx1, AF.Tanh, bias=bcol[:, 2:3])

        QR = 4 * ROWS  # quad width 2048

        def stage_eq(qd):
            base = qd * QR
            hsl = hs[:, base : base + QR]
            u = uq[qd].rearrange("p a b c -> p (a b c)")
            t = tq[qd].rearrange("p a b c -> p (a b c)")
            ua = ep.tile([P, QR], BF16, tag="ua")
            d = ep.tile([P, QR], BF16, tag="d")
            q = ep.tile([P, QR], BF16, tag="q")
            ho = ep.tile([P, QR], BF16, tag="ho")
            nc.vector.tensor_tensor(ua, u, ab[:, base : base + QR], OP.mult)
            nc.vector.tensor_tensor(d, t, hsl, OP.subtract)
            nc.vector.tensor_tensor(q, d, ua, OP.mult)
            nc.vector.tensor_tensor(ho, q, hsl, OP.add)
            nc.sync.dma_start(o_d[:, base : base + QR], ho)

        def stage_ep(p):
            base = 2 * p * ROWS
            u, t = ups[p], tps[p]
            ua = ep.tile([P, PR], BF16, tag="ua")
            d = ep.tile([P, PR], BF16, tag="d")
            q = ep.tile([P, PR], BF16, tag="q")
            ho = ep.tile([P, PR], BF16, tag="ho")
            uf = u.rearrange("p a b -> p (a b)")
            tf = t.rearrange("p a b -> p (a b)")
            if p == NP - 1:
                # final pair: per-group chains, first half drains early
                for g in range(2):
                    sl = slice(g * ROWS, (g + 1) * ROWS)
                    hgs = hs[:, base + g * ROWS : base + (g + 1) * ROWS]
                    nc.vector.tensor_tensor(ua[:, sl], uf[:, sl], ab[:, base + g * ROWS : base + (g + 1) * ROWS], OP.mult)
                    nc.vector.tensor_tensor(d[:, sl], tf[:, sl], hgs, OP.subtract)
                    nc.vector.tensor_tensor(q[:, sl], d[:, sl], ua[:, sl], OP.mult)
                    nc.vector.tensor_tensor(ho[:, sl], q[:, sl], hgs, OP.add)
                    nc.sync.dma_start(o_d[:, base + g * ROWS : base + (g + 1) * ROWS], ho[:, sl])
                return
            hsl = hs[:, base : base + PR]
            nc.vector.tensor_tensor(ua, uf, ab[:, base : base + PR], OP.mult)
            nc.vector.tensor_tensor(d, tf, hsl, OP.subtract)
            nc.vector.tensor_tensor(q, d, ua, OP.mult)
            nc.vector.tensor_tensor(ho, q, hsl, OP.add)
            nc.sync.dma_start(o_d[:, base : base + PR], ho)

        for k in range(NP + 2):
            if k < NP:
                stage_b(k)
            if 1 <= k < NP + 1:
                stage_c(k - 1)
            # quads over pairs (0,1),(2,3),(4,5): epilogue at step 2qd+3
            if k in (3, 5, 7):
                stage_eq((k - 3) // 2)
            # pairs 6,7: pair/group-granular epilogue right after stage_c
            if k in (8, 9):
                stage_ep(k - 2)

    nc.compile()
    return nc


_NC_CACHE = []


def _get_nc():
    if not _NC_CACHE:
        _NC_CACHE.append(build_program())
    return _NC_CACHE[0]


def make_in_maps(x, h_prev, att_score, W_x, b_x, W_h, b_h):
    """Shard + stage inputs for the 8 cores (bf16 wire format)."""
    x = np.asarray(x, dtype=np.float32)
    h_prev = np.asarray(h_prev, dtype=np.float32)
    att = np.asarray(att_score, dtype=np.float32)
    W_x = np.asarray(W_x, dtype=np.float32)
    W_h = np.asarray(W_h, dtype=np.float32)
    b_x = np.asarray(b_x, dtype=np.float32)
    b_h = np.asarray(b_h, dtype=np.float32)

    wxT = np.ascontiguousarray(W_x.T.reshape(I, 3, P).astype(BFNP))
    whT = np.ascontiguousarray(W_h.T.reshape(H, 3, P).astype(BFNP))
    bsum = b_x + b_h  # valid for U and R blocks
    bcol = np.stack(
        [bsum[0:P], bsum[P : 2 * P], b_x[2 * P : 3 * P], b_h[2 * P : 3 * P]], axis=1
    ).astype(np.float32)
    ident = np.eye(P, dtype=BFNP)

    in_maps = []
    for c in range(NCORES):
        s = slice(c * BL, (c + 1) * BL)
        attb = np.broadcast_to(att[s].astype(BFNP), (P, BL))
        in_maps.append(
            {
                "xT": np.ascontiguousarray(x[s].T.astype(BFNP)),
                "hT": np.ascontiguousarray(h_prev[s].T.astype(BFNP)),
                "attb": np.ascontiguousarray(attb),
                "wxT": wxT,
                "whT": whT,
                "bcol": bcol,
                "ident": ident,
            }
        )
    return in_maps


def kernel(x, h_prev, att_score, W_x, b_x, W_h, b_h, **_unused):
    nc = _get_nc()
    in_maps = make_in_maps(x, h_prev, att_score, W_x, b_x, W_h, b_h)
    res = run_bass_kernel_spmd(nc, in_maps, list(range(NCORES)))
    out = np.concatenate(
        [
            np.asarray(res.results[c]["h_newT"]).astype(np.float32).T
            for c in range(NCORES)
        ],
        axis=0,
    )
    return np.ascontiguousarray(out)


# revision 18
# speedup vs baseline: 1.1745x; 1.1745x over previous
"""AUGRU cell (attention-scaled GRU update) on 8 Trainium2 NeuronCores.

Data-parallel: batch B=65536 sharded 8 ways (8192 rows/core); gate weights
replicated.  Per core (gate-major layout, batch on the free axis):

  gates_x = x @ W_x.T + b_x
  gates_h = h @ W_h.T + b_h
  u = sigmoid(U); r = sigmoid(R); t = tanh(Cx + r*Ch)
  h_new = h + att*u*(t - h)

v12 design -- 7 matmuls/group, ACT-bias sigmoids, R-first critical path:
  - biases enter via the ACT bias operand (per-partition [P,1]) -> no K=1
    bias-prefill matmuls.  sigU/sigR are one [P,1024] ACT op per pair.
  - R gate is computed FIRST (its sigmoid feeds m -> identity-matmul ->
    tanh, the longest chain); U follows; Ch before Cx so m is never gated
    behind the pcx WAR.
  - PSUM: pu pair [P,2,512] bufs=1 (2 banks) + pr pair bufs=1 (2) +
    pcx group bufs=2 (2) + pch group bufs=2 (2) = 8 banks; split U/R pools
    release matmul WARs right after each sigmoid instead of after both.
  - tanh per group emitted in the same stage as m/id so the Cx bank WAR
    releases early; identity matmul merges m = (Ch+bCh)*r into the open
    Cx bank (216ns on PE vs ~0.75us on DVE).
  - head: first x/h slices issued from the scalar queue in parallel with
    the sync queue's weight/attb DMAs; bulk of x/h as single 1.5MiB DMAs.
  - epilogue per pair (1024 cols): ua=att*u (DVE), d=t-h (GPSIMD),
    q=ua*d (DVE), ho=h+q (DVE), one output DMA; last pair split per group.
"""

import sys

sys.path.insert(0, "/opt/trn_rl_repo")

from contextlib import ExitStack

import numpy as np
import ml_dtypes

import concourse.bass as bass
import concourse.tile as tile
from concourse import bacc, mybir
from concourse.bass_utils import run_bass_kernel_spmd

F32 = mybir.dt.float32
BF16 = mybir.dt.bfloat16
AF = mybir.ActivationFunctionType
OP = mybir.AluOpType
BFNP = ml_dtypes.bfloat16

B = 65536
NCORES = 8
BL = B // NCORES  # 8192 rows per core
I = 128
H = 128
P = 128
ROWS = 512  # batch rows per group (one fp32 PSUM bank per gate)
NGROUPS = BL // ROWS  # 16
NP = NGROUPS // 2  # 8 pairs
PR = 2 * ROWS  # pair width 1024


def build_program():
    nc = bacc.Bacc("TRN2", target_bir_lowering=False, debug=False)

    xT_d = nc.dram_tensor("xT", [I, BL], BF16, kind="ExternalInput").ap()
    hT_d = nc.dram_tensor("hT", [H, BL], BF16, kind="ExternalInput").ap()
    ab_d = nc.dram_tensor("attb", [P, BL], BF16, kind="ExternalInput").ap()
    wx_d = nc.dram_tensor("wxT", [I, 3, P], BF16, kind="ExternalInput").ap()
    wh_d = nc.dram_tensor("whT", [H, 3, P], BF16, kind="ExternalInput").ap()
    bc_d = nc.dram_tensor("bcol", [P, 4], F32, kind="ExternalInput").ap()
    id_d = nc.dram_tensor("ident", [P, P], BF16, kind="ExternalInput").ap()
    o_d = nc.dram_tensor("h_newT", [H, BL], BF16, kind="ExternalOutput").ap()

    with tile.TileContext(nc) as tc, ExitStack() as ctx:
        consts = ctx.enter_context(tc.tile_pool(name="consts", bufs=1))
        io = ctx.enter_context(tc.tile_pool(name="io", bufs=1))
        gp = ctx.enter_context(tc.tile_pool(name="gp", bufs=2))
        ep = ctx.enter_context(tc.tile_pool(name="ep", bufs=3))
        pu = ctx.enter_context(tc.tile_pool(name="pu", bufs=1, space="PSUM"))
        pr = ctx.enter_context(tc.tile_pool(name="pr", bufs=1, space="PSUM"))
        pcx = ctx.enter_context(tc.tile_pool(name="pcx", bufs=2, space="PSUM"))
        pch = ctx.enter_context(tc.tile_pool(name="pch", bufs=1, space="PSUM"))

        # ---------------- one-time setup ----------------
        # whole-core input/attb tiles; DMAs fill column ranges
        xs = io.tile([P, BL], BF16, tag="xs")
        hs = io.tile([P, BL], BF16, tag="hs")
        ab = io.tile([P, BL], BF16, tag="ab")
        wT = consts.tile([P, 6, P], BF16, tag="wT")  # [xu, xr, xc, hu, hr, hc]
        bcol = consts.tile([P, 4], F32, tag="bcol")  # [bU, bR, bCx, bCh]
        ident = consts.tile([P, P], BF16, tag="ident")

        # scalar queue carries ZERO DMAs: its first instruction is the
        # implicit ACT_TABLE_LOAD, so the first sigmoid can fire as soon as
        # the first pair's matmuls retire.
        # sync HWDGE ring is FIFO: first-pair slices + weights first, then
        # pair-granular x/h through pair 3 in consumption order.
        nc.sync.dma_start(xs[:, 0:ROWS], xT_d[:, 0:ROWS])
        nc.sync.dma_start(hs[:, 0:ROWS], hT_d[:, 0:ROWS])
        nc.sync.dma_start(wT[:, 0:3, :], wx_d)
        nc.sync.dma_start(wT[:, 3:6, :], wh_d)
        nc.sync.dma_start(bcol, bc_d)
        nc.sync.dma_start(ident, id_d)
        nc.sync.dma_start(xs[:, ROWS:PR], xT_d[:, ROWS:PR])
        nc.sync.dma_start(hs[:, ROWS:PR], hT_d[:, ROWS:PR])
        for p in range(1, 4):
            lo, hi = p * PR, (p + 1) * PR
            nc.sync.dma_start(xs[:, lo:hi], xT_d[:, lo:hi])
            nc.sync.dma_start(hs[:, lo:hi], hT_d[:, lo:hi])
        nc.sync.dma_start(ab[:, 0 : 2 * PR], ab_d[:, 0 : 2 * PR])
        nc.sync.dma_start(xs[:, 4 * PR : 6 * PR], xT_d[:, 4 * PR : 6 * PR])
        nc.sync.dma_start(hs[:, 4 * PR : 6 * PR], hT_d[:, 4 * PR : 6 * PR])
        nc.sync.dma_start(ab[:, 2 * PR : 4 * PR], ab_d[:, 2 * PR : 4 * PR])
        nc.sync.dma_start(xs[:, 6 * PR :], xT_d[:, 6 * PR :])
        nc.sync.dma_start(hs[:, 6 * PR :], hT_d[:, 6 * PR :])
        nc.sync.dma_start(ab[:, 4 * PR :], ab_d[:, 4 * PR :])

        stB = [None] * NP  # (u_ps, r_ps, cx0, cx1, ch0, ch1) per pair
        ups = [None] * NP
        tps = [None] * NP
        uas = [None] * NP

        def stage_b(p):
            sl0 = slice(2 * p * ROWS, (2 * p + 1) * ROWS)
            sl1 = slice((2 * p + 1) * ROWS, (2 * p + 2) * ROWS)
            xg = (xs[:, sl0], xs[:, sl1])
            hg = (hs[:, sl0], hs[:, sl1])
            u_ps = pu.tile([P, 2, ROWS], F32, tag="u_ps")
            r_ps = pr.tile([P, 2, ROWS], F32, tag="r_ps")
            # R first: its sigmoid heads the m -> id -> tanh chain
            nc.tensor.matmul(r_ps[:, 0, :], lhsT=wT[:, 1, :], rhs=xg[0], start=True, stop=False)
            nc.tensor.matmul(r_ps[:, 1, :], lhsT=wT[:, 1, :], rhs=xg[1], start=True, stop=False)
            nc.tensor.matmul(r_ps[:, 0, :], lhsT=wT[:, 4, :], rhs=hg[0], start=False, stop=True)
            nc.tensor.matmul(r_ps[:, 1, :], lhsT=wT[:, 4, :], rhs=hg[1], start=False, stop=True)
            nc.tensor.matmul(u_ps[:, 0, :], lhsT=wT[:, 0, :], rhs=xg[0], start=True, stop=False)
            nc.tensor.matmul(u_ps[:, 1, :], lhsT=wT[:, 0, :], rhs=xg[1], start=True, stop=False)
            nc.tensor.matmul(u_ps[:, 0, :], lhsT=wT[:, 3, :], rhs=hg[0], start=False, stop=True)
            nc.tensor.matmul(u_ps[:, 1, :], lhsT=wT[:, 3, :], rhs=hg[1], start=False, stop=True)
            ch = pch.tile([P, 2, ROWS], F32, tag="ch")
            cx0 = pcx.tile([P, ROWS], F32, tag="cx")
            cx1 = pcx.tile([P, ROWS], F32, tag="cx")
            nc.tensor.matmul(ch[:, 0, :], lhsT=wT[:, 5, :], rhs=hg[0], start=True, stop=True)
            nc.tensor.matmul(ch[:, 1, :], lhsT=wT[:, 5, :], rhs=hg[1], start=True, stop=True)
            nc.tensor.matmul(cx0, lhsT=wT[:, 2, :], rhs=xg[0], start=True, stop=False)  # stays open
            nc.tensor.matmul(cx1, lhsT=wT[:, 2, :], rhs=xg[1], start=True, stop=False)
            stB[p] = (u_ps, r_ps, cx0, cx1, ch)

        uq = [None] * (NP // 2)  # u quad tiles [P, 2(pair), 2(g), ROWS]
        tq = [None] * (NP // 2)

        def stage_c(p):
            u_ps, r_ps, cx0, cx1, ch = stB[p]
            qd, ph = p // 2, p % 2
            if p >= NP - 2:
                # last two pairs drain at pair/group granularity
                u = gp.tile([P, 2, ROWS], BF16, tag="u", name="upair")
                t = gp.tile([P, 2, ROWS], BF16, tag="t", name="tpair")
                uq[qd] = None
                ups[p], tps[p] = u, t
            else:
                if ph == 0:
                    uq[qd] = gp.tile([P, 2, 2, ROWS], BF16, tag="u", name="uquad")
                    tq[qd] = gp.tile([P, 2, 2, ROWS], BF16, tag="t", name="tquad")
                u, t = uq[qd][:, ph, :, :], tq[qd][:, ph, :, :]
            r = gp.tile([P, 2, ROWS], BF16, tag="r")
            m = gp.tile([P, 2, ROWS], BF16, tag="m")
            nc.scalar.activation(r, r_ps, AF.Sigmoid, bias=bcol[:, 1:2])
            # m per group: shortens the sigR -> m -> id -> tanh_g0 chain so
            # tanh_g0 is ready right as sigU retires (zero ACT bubble)
            nc.vector.scalar_tensor_tensor(
                m[:, 0, :], in0=ch[:, 0, :], scalar=bcol[:, 3:4], in1=r[:, 0, :],
                op0=OP.add, op1=OP.mult,
            )
            nc.tensor.matmul(cx0, lhsT=ident, rhs=m[:, 0, :], start=False, stop=True)
            nc.vector.scalar_tensor_tensor(
                m[:, 1, :], in0=ch[:, 1, :], scalar=bcol[:, 3:4], in1=r[:, 1, :],
                op0=OP.add, op1=OP.mult,
            )
            nc.tensor.matmul(cx1, lhsT=ident, rhs=m[:, 1, :], start=False, stop=True)
            nc.scalar.activation(u, u_ps, AF.Sigmoid, bias=bcol[:, 0:1])
            nc.scalar.activation(t[:, 0, :], cx0, AF.Tanh, bias=bcol[:, 2:3])
            nc.scalar.activation(t[:, 1, :], cx1, AF.Tanh, bias=bcol[:, 2:3])

        QR = 4 * ROWS  # quad width 2048

        def stage_eq(qd):
            base = qd * QR
            hsl = hs[:, base : base + QR]
            u = uq[qd].rearrange("p a b c -> p (a b c)")
            t = tq[qd].rearrange("p a b c -> p (a b c)")
            ua = ep.tile([P, QR], BF16, tag="ua")
            d = ep.tile([P, QR], BF16, tag="d")
            q = ep.tile([P, QR], BF16, tag="q")
            ho = ep.tile([P, QR], BF16, tag="ho")
            nc.vector.tensor_tensor(ua, u, ab[:, base : base + QR], OP.mult)
            nc.vector.tensor_tensor(d, t, hsl, OP.subtract)
            nc.vector.tensor_tensor(q, d, ua, OP.mult)
            nc.vector.tensor_tensor(ho, q, hsl, OP.add)
            nc.sync.dma_start(o_d[:, base : base + QR], ho)

        def stage_ep(p):
            base = 2 * p * ROWS
            u, t = ups[p], tps[p]
            ua = ep.tile([P, PR], BF16, tag="ua")
            d = ep.tile([P, PR], BF16, tag="d")
            q = ep.tile([P, PR], BF16, tag="q")
            ho = ep.tile([P, PR], BF16, tag="ho")
            uf = u.rearrange("p a b -> p (a b)")
            tf = t.rearrange("p a b -> p (a b)")
            if p == NP - 1:
                # final pair: per-group chains, first half drains early
                for g in range(2):
                    sl = slice(g * ROWS, (g + 1) * ROWS)
                    hgs = hs[:, base + g * ROWS : base + (g + 1) * ROWS]
                    nc.vector.tensor_tensor(ua[:, sl], uf[:, sl], ab[:, base + g * ROWS : base + (g + 1) * ROWS], OP.mult)
                    nc.vector.tensor_tensor(d[:, sl], tf[:, sl], hgs, OP.subtract)
                    nc.vector.tensor_tensor(q[:, sl], d[:, sl], ua[:, sl], OP.mult)
                    nc.vector.tensor_tensor(ho[:, sl], q[:, sl], hgs, OP.add)
                    nc.sync.dma_start(o_d[:, base + g * ROWS : base + (g + 1) * ROWS], ho[:, sl])
                return
            hsl = hs[:, base : base + PR]
            nc.vector.tensor_tensor(ua, uf, ab[:, base : base + PR], OP.mult)
            nc.vector.tensor_tensor(d, tf, hsl, OP.subtract)
            nc.vector.tensor_tensor(q, d, ua, OP.mult)
            nc.vector.tensor_tensor(ho, q, hsl, OP.add)
            nc.sync.dma_start(o_d[:, base : base + PR], ho)

        for k in range(NP + 2):
            if k < NP:
                stage_b(k)
            if 1 <= k < NP + 1:
                stage_c(k - 1)
            # quads over pairs (0,1),(2,3),(4,5): epilogue at step 2qd+3
            if k in (3, 5, 7):
                stage_eq((k - 3) // 2)
            # pairs 6,7: pair/group-granular epilogue right after stage_c
            if k in (8, 9):
                stage_ep(k - 2)

    nc.compile()
    return nc


_NC_CACHE = []


def _get_nc():
    if not _NC_CACHE:
        _NC_CACHE.append(build_program())
    return _NC_CACHE[0]


def make_in_maps(x, h_prev, att_score, W_x, b_x, W_h, b_h):
    """Shard + stage inputs for the 8 cores (bf16 wire format)."""
    x = np.asarray(x, dtype=np.float32)
    h_prev = np.asarray(h_prev, dtype=np.float32)
    att = np.asarray(att_score, dtype=np.float32)
    W_x = np.asarray(W_x, dtype=np.float32)
    W_h = np.asarray(W_h, dtype=np.float32)
    b_x = np.asarray(b_x, dtype=np.float32)
    b_h = np.asarray(b_h, dtype=np.float32)

    wxT = np.ascontiguousarray(W_x.T.reshape(I, 3, P).astype(BFNP))
    whT = np.ascontiguousarray(W_h.T.reshape(H, 3, P).astype(BFNP))
    bsum = b_x + b_h  # valid for U and R blocks
    bcol = np.stack(
        [bsum[0:P], bsum[P : 2 * P], b_x[2 * P : 3 * P], b_h[2 * P : 3 * P]], axis=1
    ).astype(np.float32)
    ident = np.eye(P, dtype=BFNP)

    in_maps = []
    for c in range(NCORES):
        s = slice(c * BL, (c + 1) * BL)
        attb = np.broadcast_to(att[s].astype(BFNP), (P, BL))
        in_maps.append(
            {
                "xT": np.ascontiguousarray(x[s].T.astype(BFNP)),
                "hT": np.ascontiguousarray(h_prev[s].T.astype(BFNP)),
                "attb": np.ascontiguousarray(attb),
                "wxT": wxT,
                "whT": whT,
                "bcol": bcol,
                "ident": ident,
            }
        )
    return in_maps


def kernel(x, h_prev, att_score, W_x, b_x, W_h, b_h, **_unused):
    nc = _get_nc()
    in_maps = make_in_maps(x, h_prev, att_score, W_x, b_x, W_h, b_h)
    res = run_bass_kernel_spmd(nc, in_maps, list(range(NCORES)))
    out = np.concatenate(
        [
            np.asarray(res.results[c]["h_newT"]).astype(np.float32).T
            for c in range(NCORES)
        ],
        axis=0,
    )
    return np.ascontiguousarray(out)


# revision 19
# speedup vs baseline: 1.2091x; 1.0295x over previous
"""AUGRU cell (attention-scaled GRU update) on 8 Trainium2 NeuronCores.

Data-parallel: batch B=65536 sharded 8 ways (8192 rows/core); gate weights
replicated.  Per core (gate-major layout, batch on the free axis):

  gates_x = x @ W_x.T + b_x
  gates_h = h @ W_h.T + b_h
  u = sigmoid(U); r = sigmoid(R); t = tanh(Cx + r*Ch)
  h_new = h + att*u*(t - h)

v12 design -- 7 matmuls/group, ACT-bias sigmoids, R-first critical path:
  - biases enter via the ACT bias operand (per-partition [P,1]) -> no K=1
    bias-prefill matmuls.  sigU/sigR are one [P,1024] ACT op per pair.
  - R gate is computed FIRST (its sigmoid feeds m -> identity-matmul ->
    tanh, the longest chain); U follows; Ch before Cx so m is never gated
    behind the pcx WAR.
  - PSUM: pu pair [P,2,512] bufs=1 (2 banks) + pr pair bufs=1 (2) +
    pcx group bufs=2 (2) + pch group bufs=2 (2) = 8 banks; split U/R pools
    release matmul WARs right after each sigmoid instead of after both.
  - tanh per group emitted in the same stage as m/id so the Cx bank WAR
    releases early; identity matmul merges m = (Ch+bCh)*r into the open
    Cx bank (216ns on PE vs ~0.75us on DVE).
  - head: first x/h slices issued from the scalar queue in parallel with
    the sync queue's weight/attb DMAs; bulk of x/h as single 1.5MiB DMAs.
  - epilogue per pair (1024 cols): ua=att*u (DVE), d=t-h (GPSIMD),
    q=ua*d (DVE), ho=h+q (DVE), one output DMA; last pair split per group.
"""

import sys

sys.path.insert(0, "/opt/trn_rl_repo")

from contextlib import ExitStack

import numpy as np
import ml_dtypes

import concourse.bass as bass
import concourse.tile as tile
from concourse import bacc, mybir
from concourse.bass_utils import run_bass_kernel_spmd

F32 = mybir.dt.float32
BF16 = mybir.dt.bfloat16
AF = mybir.ActivationFunctionType
OP = mybir.AluOpType
BFNP = ml_dtypes.bfloat16

B = 65536
NCORES = 8
BL = B // NCORES  # 8192 rows per core
I = 128
H = 128
P = 128
ROWS = 512  # batch rows per group (one fp32 PSUM bank per gate)
NGROUPS = BL // ROWS  # 16
NP = NGROUPS // 2  # 8 pairs
PR = 2 * ROWS  # pair width 1024


def build_program():
    nc = bacc.Bacc("TRN2", target_bir_lowering=False, debug=False)

    xT_d = nc.dram_tensor("xT", [I, BL], BF16, kind="ExternalInput").ap()
    hT_d = nc.dram_tensor("hT", [H, BL], BF16, kind="ExternalInput").ap()
    ab_d = nc.dram_tensor("attb", [P, BL], BF16, kind="ExternalInput").ap()
    wx_d = nc.dram_tensor("wxT", [I, 3, P], BF16, kind="ExternalInput").ap()
    wh_d = nc.dram_tensor("whT", [H, 3, P], BF16, kind="ExternalInput").ap()
    bc_d = nc.dram_tensor("bcol", [P, 4], F32, kind="ExternalInput").ap()
    id_d = nc.dram_tensor("ident", [P, P], BF16, kind="ExternalInput").ap()
    o_d = nc.dram_tensor("h_newT", [H, BL], BF16, kind="ExternalOutput").ap()

    with tile.TileContext(nc) as tc, ExitStack() as ctx:
        consts = ctx.enter_context(tc.tile_pool(name="consts", bufs=1))
        io = ctx.enter_context(tc.tile_pool(name="io", bufs=1))
        gp = ctx.enter_context(tc.tile_pool(name="gp", bufs=2))
        ep = ctx.enter_context(tc.tile_pool(name="ep", bufs=3))
        pu = ctx.enter_context(tc.tile_pool(name="pu", bufs=1, space="PSUM"))
        pr = ctx.enter_context(tc.tile_pool(name="pr", bufs=1, space="PSUM"))
        pcx = ctx.enter_context(tc.tile_pool(name="pcx", bufs=2, space="PSUM"))
        pch = ctx.enter_context(tc.tile_pool(name="pch", bufs=1, space="PSUM"))

        # ---------------- one-time setup ----------------
        # whole-core input/attb tiles; DMAs fill column ranges
        xs = io.tile([P, BL], BF16, tag="xs")
        hs = io.tile([P, BL], BF16, tag="hs")
        ab = io.tile([P, BL], BF16, tag="ab")
        wT = consts.tile([P, 6, P], BF16, tag="wT")  # [xu, xr, xc, hu, hr, hc]
        bcol = consts.tile([P, 4], F32, tag="bcol")  # [bU, bR, bCx, bCh]
        ident = consts.tile([P, P], BF16, tag="ident")

        # scalar queue carries ZERO DMAs: its first instruction is the
        # implicit ACT_TABLE_LOAD, so the first sigmoid can fire as soon as
        # the first pair's matmuls retire.
        # sync HWDGE ring is FIFO: first-pair slices + weights first, then
        # pair-granular x/h through pair 3 in consumption order.
        # bcol first: it gates the ACT_TABLE_LOAD that gates the first sigmoid
        nc.sync.dma_start(bcol, bc_d)
        nc.sync.dma_start(wT[:, 0:3, :], wx_d)
        nc.sync.dma_start(wT[:, 3:6, :], wh_d)
        nc.sync.dma_start(xs[:, 0:ROWS], xT_d[:, 0:ROWS])
        nc.sync.dma_start(hs[:, 0:ROWS], hT_d[:, 0:ROWS])
        nc.sync.dma_start(xs[:, ROWS:PR], xT_d[:, ROWS:PR])
        nc.sync.dma_start(hs[:, ROWS:PR], hT_d[:, ROWS:PR])
        nc.sync.dma_start(ident, id_d)
        for p in range(1, 4):
            lo, hi = p * PR, (p + 1) * PR
            nc.sync.dma_start(xs[:, lo:hi], xT_d[:, lo:hi])
            nc.sync.dma_start(hs[:, lo:hi], hT_d[:, lo:hi])
        nc.sync.dma_start(ab[:, 0 : 2 * PR], ab_d[:, 0 : 2 * PR])
        nc.sync.dma_start(xs[:, 4 * PR : 6 * PR], xT_d[:, 4 * PR : 6 * PR])
        nc.sync.dma_start(hs[:, 4 * PR : 6 * PR], hT_d[:, 4 * PR : 6 * PR])
        nc.sync.dma_start(ab[:, 2 * PR : 4 * PR], ab_d[:, 2 * PR : 4 * PR])
        nc.sync.dma_start(xs[:, 6 * PR :], xT_d[:, 6 * PR :])
        nc.sync.dma_start(hs[:, 6 * PR :], hT_d[:, 6 * PR :])
        nc.sync.dma_start(ab[:, 4 * PR :], ab_d[:, 4 * PR :])

        stB = [None] * NP  # (u_ps, r_ps, cx0, cx1, ch0, ch1) per pair
        ups = [None] * NP
        tps = [None] * NP
        uas = [None] * NP

        def stage_b(p):
            sl0 = slice(2 * p * ROWS, (2 * p + 1) * ROWS)
            sl1 = slice((2 * p + 1) * ROWS, (2 * p + 2) * ROWS)
            xg = (xs[:, sl0], xs[:, sl1])
            hg = (hs[:, sl0], hs[:, sl1])
            u_ps = pu.tile([P, 2, ROWS], F32, tag="u_ps")
            r_ps = pr.tile([P, 2, ROWS], F32, tag="r_ps")
            # R first: its sigmoid heads the m -> id -> tanh chain
            nc.tensor.matmul(r_ps[:, 0, :], lhsT=wT[:, 1, :], rhs=xg[0], start=True, stop=False)
            nc.tensor.matmul(r_ps[:, 1, :], lhsT=wT[:, 1, :], rhs=xg[1], start=True, stop=False)
            nc.tensor.matmul(r_ps[:, 0, :], lhsT=wT[:, 4, :], rhs=hg[0], start=False, stop=True)
            nc.tensor.matmul(r_ps[:, 1, :], lhsT=wT[:, 4, :], rhs=hg[1], start=False, stop=True)
            nc.tensor.matmul(u_ps[:, 0, :], lhsT=wT[:, 0, :], rhs=xg[0], start=True, stop=False)
            nc.tensor.matmul(u_ps[:, 1, :], lhsT=wT[:, 0, :], rhs=xg[1], start=True, stop=False)
            nc.tensor.matmul(u_ps[:, 0, :], lhsT=wT[:, 3, :], rhs=hg[0], start=False, stop=True)
            nc.tensor.matmul(u_ps[:, 1, :], lhsT=wT[:, 3, :], rhs=hg[1], start=False, stop=True)
            ch = pch.tile([P, 2, ROWS], F32, tag="ch")
            cx0 = pcx.tile([P, ROWS], F32, tag="cx")
            cx1 = pcx.tile([P, ROWS], F32, tag="cx")
            nc.tensor.matmul(ch[:, 0, :], lhsT=wT[:, 5, :], rhs=hg[0], start=True, stop=True)
            nc.tensor.matmul(ch[:, 1, :], lhsT=wT[:, 5, :], rhs=hg[1], start=True, stop=True)
            nc.tensor.matmul(cx0, lhsT=wT[:, 2, :], rhs=xg[0], start=True, stop=False)  # stays open
            nc.tensor.matmul(cx1, lhsT=wT[:, 2, :], rhs=xg[1], start=True, stop=False)
            stB[p] = (u_ps, r_ps, cx0, cx1, ch)

        uq = [None] * (NP // 2)  # u quad tiles [P, 2(pair), 2(g), ROWS]
        tq = [None] * (NP // 2)

        def stage_c(p):
            u_ps, r_ps, cx0, cx1, ch = stB[p]
            qd, ph = p // 2, p % 2
            if p >= NP - 2:
                # last two pairs drain at pair/group granularity
                u = gp.tile([P, 2, ROWS], BF16, tag="u", name="upair")
                t = gp.tile([P, 2, ROWS], BF16, tag="t", name="tpair")
                uq[qd] = None
                ups[p], tps[p] = u, t
            else:
                if ph == 0:
                    uq[qd] = gp.tile([P, 2, 2, ROWS], BF16, tag="u", name="uquad")
                    tq[qd] = gp.tile([P, 2, 2, ROWS], BF16, tag="t", name="tquad")
                u, t = uq[qd][:, ph, :, :], tq[qd][:, ph, :, :]
            r = gp.tile([P, 2, ROWS], BF16, tag="r")
            m = gp.tile([P, 2, ROWS], BF16, tag="m")
            nc.scalar.activation(r, r_ps, AF.Sigmoid, bias=bcol[:, 1:2])
            # m per group: shortens the sigR -> m -> id -> tanh_g0 chain so
            # tanh_g0 is ready right as sigU retires (zero ACT bubble)
            nc.vector.scalar_tensor_tensor(
                m[:, 0, :], in0=ch[:, 0, :], scalar=bcol[:, 3:4], in1=r[:, 0, :],
                op0=OP.add, op1=OP.mult,
            )
            nc.tensor.matmul(cx0, lhsT=ident, rhs=m[:, 0, :], start=False, stop=True)
            nc.vector.scalar_tensor_tensor(
                m[:, 1, :], in0=ch[:, 1, :], scalar=bcol[:, 3:4], in1=r[:, 1, :],
                op0=OP.add, op1=OP.mult,
            )
            nc.tensor.matmul(cx1, lhsT=ident, rhs=m[:, 1, :], start=False, stop=True)
            nc.scalar.activation(u, u_ps, AF.Sigmoid, bias=bcol[:, 0:1])
            nc.scalar.activation(t[:, 0, :], cx0, AF.Tanh, bias=bcol[:, 2:3])
            nc.scalar.activation(t[:, 1, :], cx1, AF.Tanh, bias=bcol[:, 2:3])

        QR = 4 * ROWS  # quad width 2048

        def stage_eq(qd):
            base = qd * QR
            hsl = hs[:, base : base + QR]
            u = uq[qd].rearrange("p a b c -> p (a b c)")
            t = tq[qd].rearrange("p a b c -> p (a b c)")
            ua = ep.tile([P, QR], BF16, tag="ua")
            d = ep.tile([P, QR], BF16, tag="d")
            q = ep.tile([P, QR], BF16, tag="q")
            ho = ep.tile([P, QR], BF16, tag="ho")
            nc.vector.tensor_tensor(ua, u, ab[:, base : base + QR], OP.mult)
            nc.vector.tensor_tensor(d, t, hsl, OP.subtract)
            nc.vector.tensor_tensor(q, d, ua, OP.mult)
            nc.vector.tensor_tensor(ho, q, hsl, OP.add)
            nc.sync.dma_start(o_d[:, base : base + QR], ho)

        def stage_ep(p):
            base = 2 * p * ROWS
            u, t = ups[p], tps[p]
            ua = ep.tile([P, PR], BF16, tag="ua")
            d = ep.tile([P, PR], BF16, tag="d")
            q = ep.tile([P, PR], BF16, tag="q")
            ho = ep.tile([P, PR], BF16, tag="ho")
            uf = u.rearrange("p a b -> p (a b)")
            tf = t.rearrange("p a b -> p (a b)")
            if p == NP - 1:
                # final pair: per-group chains, first half drains early
                for g in range(2):
                    sl = slice(g * ROWS, (g + 1) * ROWS)
                    hgs = hs[:, base + g * ROWS : base + (g + 1) * ROWS]
                    nc.vector.tensor_tensor(ua[:, sl], uf[:, sl], ab[:, base + g * ROWS : base + (g + 1) * ROWS], OP.mult)
                    nc.vector.tensor_tensor(d[:, sl], tf[:, sl], hgs, OP.subtract)
                    nc.vector.tensor_tensor(q[:, sl], d[:, sl], ua[:, sl], OP.mult)
                    nc.vector.tensor_tensor(ho[:, sl], q[:, sl], hgs, OP.add)
                    nc.sync.dma_start(o_d[:, base + g * ROWS : base + (g + 1) * ROWS], ho[:, sl])
                return
            hsl = hs[:, base : base + PR]
            nc.vector.tensor_tensor(ua, uf, ab[:, base : base + PR], OP.mult)
            nc.vector.tensor_tensor(d, tf, hsl, OP.subtract)
            nc.vector.tensor_tensor(q, d, ua, OP.mult)
            nc.vector.tensor_tensor(ho, q, hsl, OP.add)
            nc.sync.dma_start(o_d[:, base : base + PR], ho)

        for k in range(NP + 2):
            if k < NP:
                stage_b(k)
            if 1 <= k < NP + 1:
                stage_c(k - 1)
            # quads over pairs (0,1),(2,3),(4,5): epilogue at step 2qd+3
            if k in (3, 5, 7):
                stage_eq((k - 3) // 2)
            # pairs 6,7: pair/group-granular epilogue right after stage_c
            if k in (8, 9):
                stage_ep(k - 2)

    nc.compile()
    return nc


_NC_CACHE = []


def _get_nc():
    if not _NC_CACHE:
        _NC_CACHE.append(build_program())
    return _NC_CACHE[0]


def make_in_maps(x, h_prev, att_score, W_x, b_x, W_h, b_h):
    """Shard + stage inputs for the 8 cores (bf16 wire format)."""
    x = np.asarray(x, dtype=np.float32)
    h_prev = np.asarray(h_prev, dtype=np.float32)
    att = np.asarray(att_score, dtype=np.float32)
    W_x = np.asarray(W_x, dtype=np.float32)
    W_h = np.asarray(W_h, dtype=np.float32)
    b_x = np.asarray(b_x, dtype=np.float32)
    b_h = np.asarray(b_h, dtype=np.float32)

    wxT = np.ascontiguousarray(W_x.T.reshape(I, 3, P).astype(BFNP))
    whT = np.ascontiguousarray(W_h.T.reshape(H, 3, P).astype(BFNP))
    bsum = b_x + b_h  # valid for U and R blocks
    bcol = np.stack(
        [bsum[0:P], bsum[P : 2 * P], b_x[2 * P : 3 * P], b_h[2 * P : 3 * P]], axis=1
    ).astype(np.float32)
    ident = np.eye(P, dtype=BFNP)

    in_maps = []
    for c in range(NCORES):
        s = slice(c * BL, (c + 1) * BL)
        attb = np.broadcast_to(att[s].astype(BFNP), (P, BL))
        in_maps.append(
            {
                "xT": np.ascontiguousarray(x[s].T.astype(BFNP)),
                "hT": np.ascontiguousarray(h_prev[s].T.astype(BFNP)),
                "attb": np.ascontiguousarray(attb),
                "wxT": wxT,
                "whT": whT,
                "bcol": bcol,
                "ident": ident,
            }
        )
    return in_maps


def kernel(x, h_prev, att_score, W_x, b_x, W_h, b_h, **_unused):
    nc = _get_nc()
    in_maps = make_in_maps(x, h_prev, att_score, W_x, b_x, W_h, b_h)
    res = run_bass_kernel_spmd(nc, in_maps, list(range(NCORES)))
    out = np.concatenate(
        [
            np.asarray(res.results[c]["h_newT"]).astype(np.float32).T
            for c in range(NCORES)
        ],
        axis=0,
    )
    return np.ascontiguousarray(out)
